# revision 8
# baseline (speedup 1.0000x reference)
"""Causal self-attention (B=2, S=2048, E=1024, H=16, DH=64) on 8 trn2 cores.

Sharding: core c -> (batch b = c//4, head-group g = c%4, heads 4g..4g+3).
Data parallel over batch, tensor parallel over heads, row-sharded Wo;
partial outputs summed on host.

Per-core device kernel (bf16 matmuls, fp32 accumulation):
  phase A: kqT = (x @ Wkq)^T via W-stationary matmuls on xT (+bias on DVE),
           v   =  x @ Wv  (natural layout, +ones column for row-sums)
  phase B: scores^T[sk,sq] = k q^T (2 heads row-packed in PE, K=64),
           P^T = exp(scores/8) (ACT only; causal-trimmed, triu-masked diag),
           AV: saT_aug = v_aug^T @ P^T (row 64 = softmax row-sums via the
           ones column); saT = (P^T V) * (1/rowsum) fused in the PSUM
           eviction (rowsum reciprocal broadcast via GPSIMD).
  phase C: out = saT^T @ Wo (row-shard), DMA to DRAM.
"""
import numpy as np
import ml_dtypes

import concourse.bass as bass
import concourse.bacc as bacc
import concourse.tile as tile
from concourse import mybir
from concourse.masks import make_upper_triangular

BF16 = mybir.dt.bfloat16
F32 = mybir.dt.float32
NP_BF16 = ml_dtypes.bfloat16

B, S, E, H, DH = 2, 2048, 1024, 16, 64
N_CORES = 8
HPC = 4          # heads per core
SCH = 4          # number of 512-wide sq chunks
SKT = 16         # number of 128-wide sk tiles
ET = 8           # number of 128-wide e tiles

Exp = mybir.ActivationFunctionType.Exp


def build_nc(reps=1):
    nc = bacc.Bacc(None, target_bir_lowering=False)

    xT = nc.dram_tensor("xT", [E, S], BF16, kind="ExternalInput")
    wkq = nc.dram_tensor("wkq", [E, 512], BF16, kind="ExternalInput")
    wv = nc.dram_tensor("wv", [E, 256], BF16, kind="ExternalInput")
    wo = nc.dram_tensor("wo", [256, E], BF16, kind="ExternalInput")
    bkq = nc.dram_tensor("bkq", [128, 4], F32, kind="ExternalInput")
    out = nc.dram_tensor("out", [S, E], F32, kind="ExternalOutput")

    with tile.TileContext(nc) as tc:
        import contextlib
        with contextlib.ExitStack() as ctx:
            const = ctx.enter_context(tc.tile_pool(name="const", bufs=1))
            wpool = ctx.enter_context(tc.tile_pool(name="wpool", bufs=1))
            xpool = ctx.enter_context(tc.tile_pool(name="xpool", bufs=1))
            kqpool = ctx.enter_context(tc.tile_pool(name="kqpool", bufs=1))
            vpool = ctx.enter_context(tc.tile_pool(name="vpool", bufs=1))
            sapool = ctx.enter_context(tc.tile_pool(name="sapool", bufs=1))
            pt_pool = ctx.enter_context(tc.tile_pool(name="pt", bufs=6))
            bc_pool = ctx.enter_context(tc.tile_pool(name="bc", bufs=4))
            small = ctx.enter_context(tc.tile_pool(name="small", bufs=8))
            ostage = ctx.enter_context(tc.tile_pool(name="ostage", bufs=3))

            bkq_sb = const.tile([128, 4], F32)
            nc.sync.dma_start(bkq_sb[:], bkq[:])
            triu = const.tile([128, 128], BF16)
            make_upper_triangular(nc, triu[:], val=1.0, diag=True)

            wkq_sb = wpool.tile([128, ET, 512], BF16)
            nc.sync.dma_start(wkq_sb[:], wkq.rearrange("(n p) f -> p n f", p=128))
            wv_sb = wpool.tile([128, ET, 256], BF16)
            nc.sync.dma_start(wv_sb[:], wv.rearrange("(n p) f -> p n f", p=128))
            wo_sb = wpool.tile([128, 2, E], BF16)
            nc.sync.dma_start(wo_sb[:], wo.rearrange("(n p) f -> p n f", p=128))

            xT_sb = xpool.tile([128, ET, S], BF16)
            for e in range(ET):
                nc.sync.dma_start(xT_sb[:, e, :], xT[128 * e:128 * (e + 1), :])

            kqT_sb = kqpool.tile([128, 4, S], BF16)      # blk: p0k,p0q,p1k,p1q
            vaug_sb = vpool.tile([128, SKT, HPC, 65], BF16)
            nc.vector.memset(vaug_sb[:, :, :, 64:65], 1.0)
            saT_sb = sapool.tile([128, 2, S], BF16)      # dim1: pair

            for _rep in range(reps):
              # ---------------- phase A: projections ----------------
              with (
                tc.tile_pool(name="kq_ps", bufs=3, space="PSUM") as kq_ps,
                tc.tile_pool(name="v_ps", bufs=2, space="PSUM") as v_ps,
              ):
                def kq_proj(blk):
                    for c in range(SCH):
                        ps = kq_ps.tile([128, 512], F32, tag="kqps")
                        for e in range(ET):
                            nc.tensor.matmul(
                                ps[:], wkq_sb[:, e, 128 * blk:128 * (blk + 1)],
                                xT_sb[:, e, 512 * c:512 * (c + 1)],
                                start=(e == 0), stop=(e == ET - 1))
                        nc.vector.tensor_scalar_add(
                            kqT_sb[:, blk, 512 * c:512 * (c + 1)], ps[:],
                            bkq_sb[:, blk:blk + 1])

                def v_proj(t0, t1):
                    for t in range(t0, t1):
                        ps = v_ps.tile([128, 256], F32)
                        for e in range(ET):
                            nc.tensor.matmul(
                                ps[:], xT_sb[:, e, 128 * t:128 * (t + 1)],
                                wv_sb[:, e, :],
                                start=(e == 0), stop=(e == ET - 1))
                        nc.vector.tensor_copy(
                            vaug_sb[:, t, :, 0:64],
                            ps[:].rearrange("p (h d) -> p h d", h=HPC))

                kq_proj(0)
                kq_proj(1)
                v_proj(0, 4)
                kq_proj(2)
                kq_proj(3)
                v_proj(4, 16)

              # ---------------- phase B: attention ----------------
              with (
                tc.tile_pool(name="st_ps", bufs=4, space="PSUM") as st_ps,
                tc.tile_pool(name="av_ps", bufs=2, space="PSUM") as av_ps,
              ):
                for c in range(SCH):
                    sq0 = 512 * c
                    for p in range(2):
                        kblk, qblk = 2 * p, 2 * p + 1
                        avA = av_ps.tile([65, 512], F32, tag="av")
                        avB = av_ps.tile([65, 512], F32, tag="av")
                        nj = 4 * c + 4
                        for j in range(nj):
                            r = j - 4 * c
                            diag = r >= 0
                            off = 128 * r if diag else 0
                            w = 512 - off
                            stA = st_ps.tile([128, 512], F32, tag="st")
                            stB = st_ps.tile([128, 512], F32, tag="st")
                            nc.tensor.matmul(
                                stA[:, 0:w],
                                kqT_sb[0:64, kblk, 128 * j:128 * (j + 1)],
                                kqT_sb[0:64, qblk, sq0 + off:sq0 + 512],
                                start=True, stop=True, tile_position=(0, 0))
                            nc.tensor.matmul(
                                stB[:, 0:w],
                                kqT_sb[64:128, kblk, 128 * j:128 * (j + 1)],
                                kqT_sb[64:128, qblk, sq0 + off:sq0 + 512],
                                start=True, stop=True, tile_position=(64, 0))
                            ptA = pt_pool.tile([128, 512], BF16, tag="pt")
                            ptB = pt_pool.tile([128, 512], BF16, tag="pt")
                            nc.scalar.activation(ptA[:, off:512], stA[:, 0:w],
                                                 Exp, scale=0.125)
                            nc.scalar.activation(ptB[:, off:512], stB[:, 0:w],
                                                 Exp, scale=0.125)
                            if diag:
                                nc.vector.tensor_mul(
                                    ptA[:, off:off + 128],
                                    ptA[:, off:off + 128], triu[:])
                                nc.vector.tensor_mul(
                                    ptB[:, off:off + 128],
                                    ptB[:, off:off + 128], triu[:])
                            nc.tensor.matmul(avA[:, off:512],
                                             vaug_sb[:, j, 2 * p, :],
                                             ptA[:, off:512], start=(j == 0),
                                             stop=(j == nj - 1))
                            nc.tensor.matmul(avB[:, off:512],
                                             vaug_sb[:, j, 2 * p + 1, :],
                                             ptB[:, off:512], start=(j == 0),
                                             stop=(j == nj - 1))
                        # normalization + fused eviction for (p, c)
                        for sub, av in ((0, avA), (1, avB)):
                            rs = small.tile([1, 512], F32, tag="rs")
                            nc.vector.tensor_copy(rs[0:1, :], av[64:65, :])
                            rc = small.tile([1, 512], F32, tag="rc")
                            nc.vector.reciprocal(rc[0:1, :], rs[0:1, :])
                            bc = bc_pool.tile([64, 512], F32, tag="bc")
                            nc.gpsimd.partition_broadcast(bc[:], rc[0:1, :])
                            half = slice(64 * sub, 64 * (sub + 1))
                            nc.vector.tensor_mul(
                                saT_sb[half, p, sq0:sq0 + 512],
                                av[0:64, :], bc[:, :])

              # ---------------- phase C: output projection ----------------
              with tc.tile_pool(name="o_ps", bufs=2, space="PSUM") as o_ps:
                for t in range(SKT):
                    ps = o_ps.tile([128, 1024], F32)
                    for n in range(2):
                        nc.tensor.matmul(ps[:, 512 * n:512 * (n + 1)],
                                         saT_sb[:, 0, 128 * t:128 * (t + 1)],
                                         wo_sb[:, 0, 512 * n:512 * (n + 1)],
                                         start=True, stop=False)
                        nc.tensor.matmul(ps[:, 512 * n:512 * (n + 1)],
                                         saT_sb[:, 1, 128 * t:128 * (t + 1)],
                                         wo_sb[:, 1, 512 * n:512 * (n + 1)],
                                         start=False, stop=True)
                    ot = ostage.tile([128, 1024], F32)
                    if t % 2 == 0:
                        nc.vector.tensor_copy(ot[:], ps[:])
                    else:
                        nc.scalar.copy(ot[:], ps[:])
                    nc.sync.dma_start(out[128 * t:128 * (t + 1), :], ot[:])

    nc.compile()
    return nc


_CACHE = {}


def _build_runner():
    """Build the SPMD PJRT executable once; returns a dict with a jitted fn.

    Mirrors concourse.bass2jax.run_bass_via_pjrt but hoisted so repeated
    kernel() calls reuse the traced/compiled executable. No donation: the
    kernel DMA-writes every output element, so uninitialized output buffers
    are fine.
    """
    import jax
    from jax.sharding import Mesh, PartitionSpec
    from jax.experimental.shard_map import shard_map
    from concourse import bass2jax as b2j
    from concourse import mybir as _mybir

    if "runner" in _CACHE:
        return _CACHE["runner"]

    nc = _CACHE.get("nc")
    if nc is None:
        nc = _CACHE["nc"] = build_nc()

    b2j.install_neuronx_cc_hook()
    partition_name = (nc.partition_id_tensor.name
                      if nc.partition_id_tensor else None)

    in_names, out_names, out_avals = [], [], []
    for alloc in nc.m.functions[0].allocations:
        if not isinstance(alloc, _mybir.MemoryLocationSet):
            continue
        name = alloc.memorylocations[0].name
        if alloc.kind == "ExternalInput":
            if name != partition_name:
                in_names.append(name)
        elif alloc.kind == "ExternalOutput":
            out_names.append(name)
            out_avals.append(jax.core.ShapedArray(
                tuple(alloc.tensor_shape), _mybir.dt.np(alloc.dtype)))
    n_params = len(in_names)
    zero_out_shapes = [(a.shape, a.dtype) for a in out_avals]
    all_in_names = list(in_names) + list(out_names)
    if partition_name is not None:
        all_in_names.append(partition_name)

    def _body(*args):
        operands = list(args)
        if partition_name is not None:
            operands.append(b2j.partition_id_tensor())
        outs = b2j._bass_exec_p.bind(
            *operands,
            out_avals=tuple(out_avals),
            in_names=tuple(all_in_names),
            out_names=tuple(out_names),
            lowering_input_output_aliases=(),
            sim_require_finite=True,
            sim_require_nnan=True,
            nc=nc,
        )
        return tuple(outs)

    devices = jax.devices()[:N_CORES]
    mesh = Mesh(np.asarray(devices), ("core",))
    n_outs = len(out_names)
    in_specs = (PartitionSpec("core"),) * (n_params + n_outs)
    out_specs = (PartitionSpec("core"),) * n_outs
    fn = jax.jit(shard_map(_body, mesh=mesh, in_specs=in_specs,
                           out_specs=out_specs, check_rep=False),
                 keep_unused=True)
    runner = {
        "fn": fn,
        "in_names": in_names,
        "out_names": out_names,
        "out_avals": out_avals,
        "zero_out_shapes": zero_out_shapes,
        "mesh": mesh,
    }
    _CACHE["runner"] = runner
    return runner


def _run_spmd(in_maps):
    """Execute on 8 cores, returning list of per-core output dicts."""
    r = _build_runner()
    n_cores = N_CORES
    concat_in = [
        np.concatenate([np.asarray(in_maps[c][name]) for c in range(n_cores)],
                       axis=0)
        for name in r["in_names"]
    ]
    if "zeros" not in r:
        r["zeros"] = [np.zeros((n_cores * s[0], *s[1:]), d)
                      for s, d in r["zero_out_shapes"]]
    out_arrs = r["fn"](*concat_in, *r["zeros"])
    return [
        {name: np.asarray(out_arrs[i]).reshape(n_cores, *r["out_avals"][i].shape)[c]
         for i, name in enumerate(r["out_names"])}
        for c in range(n_cores)
    ]


def _prep_core_inputs(x, Wkqv, bkqv, Wo):
    """Host-side shard/pack. Returns (in_maps, host_bias) for 8 cores."""
    xT = [np.ascontiguousarray(x[b].T).astype(NP_BF16) for b in range(B)]
    per_g = []
    for g in range(4):
        h0 = 4 * g
        wkq = np.empty((E, 512), np.float32)
        for p in range(2):
            a, b_ = h0 + 2 * p, h0 + 2 * p + 1
            wkq[:, 256 * p:256 * p + 64] = Wkqv[a][:, 0:64]
            wkq[:, 256 * p + 64:256 * p + 128] = Wkqv[b_][:, 0:64]
            wkq[:, 256 * p + 128:256 * p + 192] = Wkqv[a][:, 64:128]
            wkq[:, 256 * p + 192:256 * p + 256] = Wkqv[b_][:, 64:128]
        wv = np.concatenate([Wkqv[h0 + h][:, 128:192] for h in range(HPC)],
                            axis=1)
        wog = Wo[256 * g:256 * (g + 1), :]
        bkq_arr = np.empty((128, 4), np.float32)
        for p in range(2):
            a, b_ = h0 + 2 * p, h0 + 2 * p + 1
            bkq_arr[0:64, 2 * p] = bkqv[a][0:64]
            bkq_arr[64:128, 2 * p] = bkqv[b_][0:64]
            bkq_arr[0:64, 2 * p + 1] = bkqv[a][64:128]
            bkq_arr[64:128, 2 * p + 1] = bkqv[b_][64:128]
        per_g.append({
            "wkq": wkq.astype(NP_BF16),
            "wv": wv.astype(NP_BF16),
            "wo": wog.astype(NP_BF16),
            "bkq": bkq_arr,
        })
    in_maps = []
    for c in range(N_CORES):
        b, g = c // 4, c % 4
        m = dict(per_g[g])
        m["xT"] = xT[b]
        in_maps.append(m)
    bv = np.concatenate([bkqv[h][128:192] for h in range(H)])
    return in_maps, bv


def kernel(x, Wkqv, bkqv, Wo, bo):
    x = np.asarray(x, np.float32)
    Wkqv = np.asarray(Wkqv, np.float32)
    bkqv = np.asarray(bkqv, np.float32)
    Wo = np.asarray(Wo, np.float32)
    bo = np.asarray(bo, np.float32)

    in_maps, bv = _prep_core_inputs(x, Wkqv, bkqv, Wo)
    results = _run_spmd(in_maps)
    partials = np.stack([results[c]["out"] for c in range(N_CORES)])
    partials = partials.reshape(B, 4, S, E).sum(axis=1)
    base = bv @ Wo + bo
    return (partials + base[None, None, :]).astype(np.float32)


# revision 11
# speedup vs baseline: 123.8562x; 123.8562x over previous
"""Causal self-attention (B=2, S=2048, E=1024, H=16, DH=64) on 8 trn2 cores.

Sharding: core c -> (batch b = c//4, head-group g = c%4, heads 4g..4g+3).
Data parallel over batch, tensor parallel over heads, row-sharded Wo;
partial outputs summed on host.

Per-core device kernel (bf16 matmuls, fp32 accumulation):
  phase A: kqT = (x @ Wkq)^T via W-stationary matmuls on xT (+bias on DVE),
           v   =  x @ Wv  (natural layout, +ones column for row-sums)
  phase B: scores^T[sk,sq] = k q^T (2 heads row-packed in PE, K=64),
           P^T = exp(scores/8) (ACT only; causal-trimmed, triu-masked diag),
           AV: saT_aug = v_aug^T @ P^T (row 64 = softmax row-sums via the
           ones column); saT = (P^T V) * (1/rowsum) fused in the PSUM
           eviction (rowsum reciprocal broadcast via GPSIMD).
  phase C: out = saT^T @ Wo (row-shard), DMA to DRAM.
"""
import numpy as np
import ml_dtypes

import concourse.bass as bass
import concourse.bacc as bacc
import concourse.tile as tile
from concourse import mybir
from concourse.masks import make_upper_triangular

BF16 = mybir.dt.bfloat16
F32 = mybir.dt.float32
NP_BF16 = ml_dtypes.bfloat16

B, S, E, H, DH = 2, 2048, 1024, 16, 64
N_CORES = 8
HPC = 4          # heads per core
SCH = 4          # number of 512-wide sq chunks
SKT = 16         # number of 128-wide sk tiles
ET = 8           # number of 128-wide e tiles

Exp = mybir.ActivationFunctionType.Exp


def build_nc(reps=1):
    nc = bacc.Bacc(None, target_bir_lowering=False)

    xT = nc.dram_tensor("xT", [E, S], BF16, kind="ExternalInput")
    wkq = nc.dram_tensor("wkq", [E, 512], BF16, kind="ExternalInput")
    wv = nc.dram_tensor("wv", [E, 256], BF16, kind="ExternalInput")
    wo = nc.dram_tensor("wo", [256, E], BF16, kind="ExternalInput")
    bkq = nc.dram_tensor("bkq", [128, 4], F32, kind="ExternalInput")
    out = nc.dram_tensor("out", [S, E], F32, kind="ExternalOutput")

    with tile.TileContext(nc) as tc:
        import contextlib
        with contextlib.ExitStack() as ctx:
            const = ctx.enter_context(tc.tile_pool(name="const", bufs=1))
            wpool = ctx.enter_context(tc.tile_pool(name="wpool", bufs=1))
            xpool = ctx.enter_context(tc.tile_pool(name="xpool", bufs=1))
            kqpool = ctx.enter_context(tc.tile_pool(name="kqpool", bufs=1))
            vpool = ctx.enter_context(tc.tile_pool(name="vpool", bufs=1))
            sapool = ctx.enter_context(tc.tile_pool(name="sapool", bufs=1))
            pt_pool = ctx.enter_context(tc.tile_pool(name="pt", bufs=6))
            bc_pool = ctx.enter_context(tc.tile_pool(name="bc", bufs=4))
            small = ctx.enter_context(tc.tile_pool(name="small", bufs=8))
            ostage = ctx.enter_context(tc.tile_pool(name="ostage", bufs=3))

            bkq_sb = const.tile([128, 4], F32)
            nc.sync.dma_start(bkq_sb[:], bkq[:])
            triu2 = const.tile([128, 2, 128], BF16)
            make_upper_triangular(nc, triu2[:, 0, :], val=1.0, diag=True)
            make_upper_triangular(nc, triu2[:, 1, :], val=1.0, diag=True)

            wkq_sb = wpool.tile([128, ET, 512], BF16)
            nc.sync.dma_start(wkq_sb[:], wkq.rearrange("(n p) f -> p n f", p=128))
            wv_sb = wpool.tile([128, ET, 256], BF16)
            nc.sync.dma_start(wv_sb[:], wv.rearrange("(n p) f -> p n f", p=128))
            wo_sb = wpool.tile([128, 2, E], BF16)
            nc.sync.dma_start(wo_sb[:], wo.rearrange("(n p) f -> p n f", p=128))

            xT_sb = xpool.tile([128, ET, S], BF16)
            for e in range(ET):
                nc.sync.dma_start(xT_sb[:, e, :], xT[128 * e:128 * (e + 1), :])

            kqT_sb = kqpool.tile([128, 4, S], BF16)      # blk: p0k,p0q,p1k,p1q
            vaug_sb = vpool.tile([128, SKT, HPC, 65], BF16)
            nc.vector.memset(vaug_sb[:, :, :, 64:65], 1.0)
            saT_sb = sapool.tile([128, 2, S], BF16)      # dim1: pair

            for _rep in range(reps):
              # ---------------- phase A: projections ----------------
              with (
                tc.tile_pool(name="kq_ps", bufs=3, space="PSUM") as kq_ps,
                tc.tile_pool(name="v_ps", bufs=2, space="PSUM") as v_ps,
              ):
                def kq_proj(blk):
                    for c in range(SCH):
                        ps = kq_ps.tile([128, 512], F32, tag="kqps")
                        for e in range(ET):
                            nc.tensor.matmul(
                                ps[:], wkq_sb[:, e, 128 * blk:128 * (blk + 1)],
                                xT_sb[:, e, 512 * c:512 * (c + 1)],
                                start=(e == 0), stop=(e == ET - 1))
                        nc.vector.tensor_scalar_add(
                            kqT_sb[:, blk, 512 * c:512 * (c + 1)], ps[:],
                            bkq_sb[:, blk:blk + 1])

                def v_proj(t0, t1):
                    for t in range(t0, t1):
                        ps = v_ps.tile([128, 256], F32)
                        for e in range(ET):
                            nc.tensor.matmul(
                                ps[:], xT_sb[:, e, 128 * t:128 * (t + 1)],
                                wv_sb[:, e, :],
                                start=(e == 0), stop=(e == ET - 1))
                        nc.vector.tensor_copy(
                            vaug_sb[:, t, :, 0:64],
                            ps[:].rearrange("p (h d) -> p h d", h=HPC))

                kq_proj(0)
                kq_proj(1)
                v_proj(0, 4)
                kq_proj(2)
                kq_proj(3)
                v_proj(4, 16)

              # ---------------- phase B: attention ----------------
              with (
                tc.tile_pool(name="st_ps", bufs=2, space="PSUM") as st_ps,
                tc.tile_pool(name="av_ps", bufs=2, space="PSUM") as av_ps,
              ):
                for c in range(SCH):
                    sq0 = 512 * c
                    for p in range(2):
                        kblk, qblk = 2 * p, 2 * p + 1
                        av = av_ps.tile([65, 2, 512], F32, tag="av")
                        nj = 4 * c + 4
                        for j in range(nj):
                            r = j - 4 * c
                            diag = r >= 0
                            off = 128 * r if diag else 0
                            w = 512 - off
                            st = st_ps.tile([128, 2, 512], F32, tag="st")
                            nc.tensor.matmul(
                                st[:, 0, 0:w],
                                kqT_sb[0:64, kblk, 128 * j:128 * (j + 1)],
                                kqT_sb[0:64, qblk, sq0 + off:sq0 + 512],
                                start=True, stop=True, tile_position=(0, 0))
                            nc.tensor.matmul(
                                st[:, 1, 0:w],
                                kqT_sb[64:128, kblk, 128 * j:128 * (j + 1)],
                                kqT_sb[64:128, qblk, sq0 + off:sq0 + 512],
                                start=True, stop=True, tile_position=(64, 0))
                            pt = pt_pool.tile([128, 2, 512], BF16, tag="pt")
                            nc.scalar.activation(pt[:, :, off:512],
                                                 st[:, :, 0:w],
                                                 Exp, scale=0.125)
                            if diag:
                                nc.vector.tensor_mul(
                                    pt[:, :, off:off + 128],
                                    pt[:, :, off:off + 128], triu2[:])
                            nc.tensor.matmul(av[:, 0, off:512],
                                             vaug_sb[:, j, 2 * p, :],
                                             pt[:, 0, off:512], start=(j == 0),
                                             stop=(j == nj - 1))
                            nc.tensor.matmul(av[:, 1, off:512],
                                             vaug_sb[:, j, 2 * p + 1, :],
                                             pt[:, 1, off:512], start=(j == 0),
                                             stop=(j == nj - 1))
                        # normalization + fused eviction for (p, c)
                        rs = small.tile([1, 2, 512], F32, tag="rs")
                        nc.vector.tensor_copy(rs[0:1, :, :], av[64:65, :, :])
                        rc = small.tile([1, 2, 512], F32, tag="rc")
                        nc.vector.reciprocal(rc[0:1, :, :], rs[0:1, :, :])
                        bc = bc_pool.tile([64, 2, 512], F32, tag="bc")
                        nc.gpsimd.partition_broadcast(bc[:], rc[0:1, :, :])
                        for sub in (0, 1):
                            half = slice(64 * sub, 64 * (sub + 1))
                            nc.vector.tensor_mul(
                                saT_sb[half, p, sq0:sq0 + 512],
                                av[0:64, sub, :], bc[:, sub, :])

              # ---------------- phase C: output projection ----------------
              with tc.tile_pool(name="o_ps", bufs=2, space="PSUM") as o_ps:
                for t in range(SKT):
                    ps = o_ps.tile([128, 1024], F32)
                    for n in range(2):
                        nc.tensor.matmul(ps[:, 512 * n:512 * (n + 1)],
                                         saT_sb[:, 0, 128 * t:128 * (t + 1)],
                                         wo_sb[:, 0, 512 * n:512 * (n + 1)],
                                         start=True, stop=False)
                        nc.tensor.matmul(ps[:, 512 * n:512 * (n + 1)],
                                         saT_sb[:, 1, 128 * t:128 * (t + 1)],
                                         wo_sb[:, 1, 512 * n:512 * (n + 1)],
                                         start=False, stop=True)
                    ot = ostage.tile([128, 1024], F32)
                    if t % 2 == 0:
                        nc.vector.tensor_copy(ot[:], ps[:])
                    else:
                        nc.scalar.copy(ot[:], ps[:])
                    nc.sync.dma_start(out[128 * t:128 * (t + 1), :], ot[:])

    nc.compile()
    return nc


_CACHE = {}


def _build_runner():
    """Build the SPMD PJRT executable once; returns a dict with a jitted fn.

    Mirrors concourse.bass2jax.run_bass_via_pjrt but hoisted so repeated
    kernel() calls reuse the traced/compiled executable. No donation: the
    kernel DMA-writes every output element, so uninitialized output buffers
    are fine.
    """
    import jax
    from jax.sharding import Mesh, PartitionSpec
    from jax.experimental.shard_map import shard_map
    from concourse import bass2jax as b2j
    from concourse import mybir as _mybir

    if "runner" in _CACHE:
        return _CACHE["runner"]

    nc = _CACHE.get("nc")
    if nc is None:
        nc = _CACHE["nc"] = build_nc()

    b2j.install_neuronx_cc_hook()
    partition_name = (nc.partition_id_tensor.name
                      if nc.partition_id_tensor else None)

    in_names, out_names, out_avals = [], [], []
    for alloc in nc.m.functions[0].allocations:
        if not isinstance(alloc, _mybir.MemoryLocationSet):
            continue
        name = alloc.memorylocations[0].name
        if alloc.kind == "ExternalInput":
            if name != partition_name:
                in_names.append(name)
        elif alloc.kind == "ExternalOutput":
            out_names.append(name)
            out_avals.append(jax.core.ShapedArray(
                tuple(alloc.tensor_shape), _mybir.dt.np(alloc.dtype)))
    n_params = len(in_names)
    zero_out_shapes = [(a.shape, a.dtype) for a in out_avals]
    all_in_names = list(in_names) + list(out_names)
    if partition_name is not None:
        all_in_names.append(partition_name)

    def _body(*args):
        operands = list(args)
        if partition_name is not None:
            operands.append(b2j.partition_id_tensor())
        outs = b2j._bass_exec_p.bind(
            *operands,
            out_avals=tuple(out_avals),
            in_names=tuple(all_in_names),
            out_names=tuple(out_names),
            lowering_input_output_aliases=(),
            sim_require_finite=True,
            sim_require_nnan=True,
            nc=nc,
        )
        return tuple(outs)

    devices = jax.devices()[:N_CORES]
    mesh = Mesh(np.asarray(devices), ("core",))
    n_outs = len(out_names)
    in_specs = (PartitionSpec("core"),) * (n_params + n_outs)
    out_specs = (PartitionSpec("core"),) * n_outs
    fn = jax.jit(shard_map(_body, mesh=mesh, in_specs=in_specs,
                           out_specs=out_specs, check_rep=False),
                 keep_unused=True)
    runner = {
        "fn": fn,
        "in_names": in_names,
        "out_names": out_names,
        "out_avals": out_avals,
        "zero_out_shapes": zero_out_shapes,
        "mesh": mesh,
    }
    _CACHE["runner"] = runner
    return runner


def _run_spmd(in_maps):
    """Execute on 8 cores, returning list of per-core output dicts."""
    r = _build_runner()
    n_cores = N_CORES
    concat_in = [
        np.concatenate([np.asarray(in_maps[c][name]) for c in range(n_cores)],
                       axis=0)
        for name in r["in_names"]
    ]
    if "zeros" not in r:
        r["zeros"] = [np.zeros((n_cores * s[0], *s[1:]), d)
                      for s, d in r["zero_out_shapes"]]
    out_arrs = r["fn"](*concat_in, *r["zeros"])
    return [
        {name: np.asarray(out_arrs[i]).reshape(n_cores, *r["out_avals"][i].shape)[c]
         for i, name in enumerate(r["out_names"])}
        for c in range(n_cores)
    ]


def _prep_core_inputs(x, Wkqv, bkqv, Wo):
    """Host-side shard/pack. Returns (in_maps, host_bias) for 8 cores."""
    xT = [np.ascontiguousarray(x[b].T).astype(NP_BF16) for b in range(B)]
    per_g = []
    for g in range(4):
        h0 = 4 * g
        wkq = np.empty((E, 512), np.float32)
        for p in range(2):
            a, b_ = h0 + 2 * p, h0 + 2 * p + 1
            wkq[:, 256 * p:256 * p + 64] = Wkqv[a][:, 0:64]
            wkq[:, 256 * p + 64:256 * p + 128] = Wkqv[b_][:, 0:64]
            wkq[:, 256 * p + 128:256 * p + 192] = Wkqv[a][:, 64:128]
            wkq[:, 256 * p + 192:256 * p + 256] = Wkqv[b_][:, 64:128]
        wv = np.concatenate([Wkqv[h0 + h][:, 128:192] for h in range(HPC)],
                            axis=1)
        wog = Wo[256 * g:256 * (g + 1), :]
        bkq_arr = np.empty((128, 4), np.float32)
        for p in range(2):
            a, b_ = h0 + 2 * p, h0 + 2 * p + 1
            bkq_arr[0:64, 2 * p] = bkqv[a][0:64]
            bkq_arr[64:128, 2 * p] = bkqv[b_][0:64]
            bkq_arr[0:64, 2 * p + 1] = bkqv[a][64:128]
            bkq_arr[64:128, 2 * p + 1] = bkqv[b_][64:128]
        per_g.append({
            "wkq": wkq.astype(NP_BF16),
            "wv": wv.astype(NP_BF16),
            "wo": wog.astype(NP_BF16),
            "bkq": bkq_arr,
        })
    in_maps = []
    for c in range(N_CORES):
        b, g = c // 4, c % 4
        m = dict(per_g[g])
        m["xT"] = xT[b]
        in_maps.append(m)
    bv = np.concatenate([bkqv[h][128:192] for h in range(H)])
    return in_maps, bv


def kernel(x, Wkqv, bkqv, Wo, bo):
    x = np.asarray(x, np.float32)
    Wkqv = np.asarray(Wkqv, np.float32)
    bkqv = np.asarray(bkqv, np.float32)
    Wo = np.asarray(Wo, np.float32)
    bo = np.asarray(bo, np.float32)

    in_maps, bv = _prep_core_inputs(x, Wkqv, bkqv, Wo)
    results = _run_spmd(in_maps)
    partials = np.stack([results[c]["out"] for c in range(N_CORES)])
    partials = partials.reshape(B, 4, S, E).sum(axis=1)
    base = bv @ Wo + bo
    return (partials + base[None, None, :]).astype(np.float32)
